# revision 20
# baseline (speedup 1.0000x reference)
"""Trainium2 Bass kernel for nn_CombinedGraphLoss (graph-loss over 8192x8192 adj).

loss = sum((A - decay)^2) + 0.1*sum|A - mean4(A)| + 0.001*sum(A^2)
with A = D^-1/2 relu(adj) D^-1/2, decay = exp(-0.1|i-j|).

Strategy (8 cores, row-sharded, full inputs per core):
  - each core gets its 1024-row shard (+1 halo row each side) as its own input
  - pass1: row sums d (relu on gpsimd, reduce on DVE)
  - AllGather d -> dinv = exp(-0.5*ln(d+eps)) (exact-ish, avoids bad Rsqrt)
  - pass2 (9 overlapping 128-row tiles, stride 126):
      A1 = Relu(adj*dinv_i) on ACT (bf16), A = A1*colfac on DVE,
      stencil t = A - 0.25*(up+down+left+right) built on PE via 3 matmuls/chunk,
      |t| row-sums via ACT Abs(accum_out) from PSUM,
      sum A^2 and band sum A*decay via DVE tensor_tensor_reduce(accum_out).
  - decay terms decomposed: sum(A-decay)^2 = sumA^2 - 2*sum(A*decay) + sum(decay^2);
    sum decay^2 is analytic on host; A*decay only inside |i-j|<=1088 band
    (decay==0 exactly in fp32 outside |i-j|>1039).
  - host applies row-ownership masks (overlap tiles) and reduces in float64.
"""

import numpy as np

import concourse.bass as bass
import concourse.mybir as mybir
from concourse import tile
from concourse.bass_utils import run_bass_kernel_spmd

try:
    from waitstrip import legalize_waits
except ImportError:  # self-contained fallback when shipped alone
    legalize_waits = None

N = 8192
NC = 8
SH = N // NC          # 1024 rows per core
LR = SH + 2           # local rows incl halos = 1026
ALPHA = 0.1
LAM = 0.1
GAMMA = 0.001

BW = 2304             # band width (covers |i-j| <= 1088 for every tile row)
PAD = 1152            # zero padding on each side of A_pad
APW = N + 2 * PAD     # 10496
CB = PAD              # first real column inside A_pad
R0S = [126 * k for k in range(8)] + [LR - 128]   # pass2 tile starts (local rows)
NT2 = len(R0S)

f32 = mybir.dt.float32
bf16 = mybir.dt.bfloat16
i32 = mybir.dt.int32
Alu = mybir.AluOpType
Act = mybir.ActivationFunctionType
X = mybir.AxisListType.X

# accumulator column layout in the [128, 64] f32 output
SM_COL = 0     # 36 cols: tile k quarter q -> 4k+q, rows 0..125
A2_COL = 36    # 9 cols: tile k, rows 0..127
BD_COL = 45    # 9 cols: tile k, rows 0..127


def _build_nc():
    nc = bass.Bass(num_devices=NC)
    adj_in = nc.dram_tensor("adj_sh", [LR, N], f32, kind="ExternalInput")
    res_out = nc.dram_tensor("res", [128, 64], f32, kind="ExternalOutput")

    with tile.TileContext(nc) as tc:
        with (
            tc.tile_pool(name="const", bufs=1) as cp,
            tc.tile_pool(name="dram", bufs=1, space="DRAM") as dram,
            tc.tile_pool(name="io", bufs=2) as iop,
            tc.tile_pool(name="a1p", bufs=2) as a1p,
            tc.tile_pool(name="apad", bufs=1) as apadp,
            tc.tile_pool(name="scr", bufs=1) as scrp,
            tc.tile_pool(name="ps", bufs=1, space="PSUM") as psp,
        ):
            acc = cp.tile([128, 64], f32)
            nc.vector.memset(acc[:], 0.0)
            epsb = cp.tile([128, 1], f32)
            nc.vector.memset(epsb[:], 1e-10)

            apads = [apadp.tile([128, APW], bf16, tag=f"apad{i}", name=f"apad{i}") for i in range(2)]
            for a_t in apads:
                nc.gpsimd.memset(a_t[:, 0:PAD], 0.0)
                nc.gpsimd.memset(a_t[:, PAD + N : APW], 0.0)
            psums = [psp.tile([128, 2048], f32, tag=f"ps{i}", name=f"ps{i}") for i in range(2)]

            # ---- stencil lhsT matrices: Mv[p,l] = d(p,l+1) -0.25 d(p,l) -0.25 d(p,l+2)
            Mv = cp.tile([128, 126], bf16)
            NI = cp.tile([128, 126], bf16)
            idx = cp.tile([128, 126], i32)
            nc.gpsimd.iota(idx[:], pattern=[[-1, 126]], base=0, channel_multiplier=1)
            idxf = cp.tile([128, 126], f32)
            nc.gpsimd.tensor_copy(idxf[:], idx[:])
            vm1 = cp.tile([128, 126], f32)
            nc.vector.tensor_scalar(vm1[:], idxf[:], 1.0, None, Alu.subtract)  # p-l-1
            vab = cp.tile([128, 126], f32)
            vneg = cp.tile([128, 126], f32)
            nc.vector.tensor_scalar(vneg[:], vm1[:], -1.0, None, Alu.mult)
            nc.vector.tensor_max(vab[:], vm1[:], vneg[:])                      # |p-l-1|
            near = cp.tile([128, 126], f32)
            nc.vector.tensor_scalar(near[:], vab[:], 1.0, None, Alu.is_le)     # |.|<=1
            ctr = cp.tile([128, 126], f32)
            nc.vector.tensor_scalar(ctr[:], vab[:], 0.0, None, Alu.is_equal)   # ==0
            near4 = cp.tile([128, 126], f32)
            nc.vector.tensor_scalar(near4[:], near[:], 0.25, None, Alu.mult)
            ctr125 = cp.tile([128, 126], f32)
            nc.vector.tensor_scalar(ctr125[:], ctr[:], 1.25, None, Alu.mult)
            nc.vector.tensor_sub(Mv[:], ctr125[:], near4[:])
            nc.vector.tensor_scalar(NI[:], ctr[:], -0.25, None, Alu.mult)

            # ---- decay band constant: D[p,u] = exp(-0.1*|1088 + p - u|)
            decayb = cp.tile([128, BW], bf16)
            bidx = scrp.tile([128, BW], i32, tag="junk", name="bidx")
            nc.gpsimd.iota(bidx[:], pattern=[[-1, BW]], base=1088, channel_multiplier=1)
            bidf = scrp.tile([128, BW], f32, tag="sabs", name="bidf")
            nc.gpsimd.tensor_copy(bidf[:], bidx[:])
            babs = scrp.tile([128, BW], f32, tag="junk", name="babs")
            bneg = a1p.tile([128, BW], f32, tag="A1", name="bneg")
            nc.vector.tensor_scalar(bneg[:], bidf[:], -1.0, None, Alu.mult)
            nc.vector.tensor_max(babs[:], bidf[:], bneg[:])
            nc.scalar.activation(decayb[:], babs[:], Act.Exp, scale=-ALPHA)

            # ---- pass 1: d = row sums of relu(adj) over all 1026 local rows
            # relu'd bf16 rows staged in the apad center (reused later by pass 2)
            d_sb = cp.tile([128, 16], f32)
            nc.vector.memset(d_sb[:], 0.0)
            for k in range(8):
                t = iop.tile([128, N], f32, tag="adj", name=f"p1t{k}")
                nc.gpsimd.dma_start(t[:], adj_in[128 * k : 128 * k + 128, :])
                rl = apads[k % 2][:, CB : CB + N]
                nc.scalar.activation(rl, t[:], Act.Relu)
                nc.vector.tensor_reduce(d_sb[:, k : k + 1], rl, axis=X, op=Alu.add)
            t9 = iop.tile([2, N], f32, tag="adj", name="p1t8")
            nc.gpsimd.dma_start(t9[:], adj_in[1024:1026, :])
            rl = apads[0][0:2, CB : CB + N]
            nc.scalar.activation(rl, t9[:], Act.Relu)
            nc.vector.tensor_reduce(d_sb[0:2, 8:9], rl, axis=X, op=Alu.add)

            # ---- dinv_local = exp(-0.5*ln(d + 1e-10)) ; store to DRAM flat [1152]
            lnd = cp.tile([128, 16], f32)
            nc.scalar.activation(lnd[:, 0:9], d_sb[:, 0:9], Act.Ln, bias=epsb[:])
            dinv_sb = cp.tile([128, 16], f32)
            nc.scalar.activation(dinv_sb[:, 0:9], lnd[:, 0:9], Act.Exp, scale=-0.5)
            dinvloc = dram.tile([1, 1152], f32)
            for k in range(9):
                nc.gpsimd.dma_start(
                    dinvloc[0:1, 128 * k : 128 * k + 128],
                    dinv_sb[:, k : k + 1],
                )

            # ---- AllGather of own d (local rows 1..1024 = global shard rows)
            dcore = dram.tile([1, SH], f32)
            nc.gpsimd.dma_start(dcore[0:1, 0:127], d_sb[1:128, 0:1])
            for k in range(1, 8):
                nc.gpsimd.dma_start(
                    dcore[0:1, 128 * k - 1 : 128 * k + 127], d_sb[:, k : k + 1]
                )
            nc.gpsimd.dma_start(dcore[0:1, 1023:1024], d_sb[0:1, 8:9])
            dglob = dram.tile([NC, SH], f32)
            nc.gpsimd.collective_compute(
                "AllGather",
                Alu.bypass,
                replica_groups=[list(range(NC))],
                ins=[dcore.opt()],
                outs=[dglob.opt()],
            )

            # ---- global dinv -> padded bf16 DRAM vector + colfac tile
            dg = cp.tile([128, 64], f32)
            nc.gpsimd.dma_start(
                dg[:], dglob[:].rearrange("a b -> (a b)").rearrange("(p t) -> p t", p=128)
            )
            lng = cp.tile([128, 64], f32)
            nc.scalar.activation(lng[:], dg[:], Act.Ln, bias=epsb[:])
            dgi = cp.tile([128, 64], f32)
            nc.scalar.activation(dgi[:], lng[:], Act.Exp, scale=-0.5)
            dgib = cp.tile([128, 64], bf16)
            nc.vector.tensor_copy(dgib[:], dgi[:])
            dinv3 = dram.tile([1, APW], bf16)
            zpad = cp.tile([1, PAD], bf16)
            nc.vector.memset(zpad[:], 0.0)
            nc.gpsimd.dma_start(dinv3[0:1, 0:PAD], zpad[0:1, :])
            nc.gpsimd.dma_start(dinv3[0:1, PAD + N : APW], zpad[0:1, :])
            nc.gpsimd.dma_start(
                dinv3[0:1, PAD : PAD + N].rearrange("o (p t) -> (o p) t", p=128),
                dgib[:],
            )
            colfac = cp.tile([128, N], bf16)
            nc.gpsimd.dma_start(colfac[0:1, :], dinv3[0:1, PAD : PAD + N])
            p = 1
            while p < 128:
                nc.gpsimd.dma_start(colfac[p : 2 * p, :], colfac[0:p, :])
                p *= 2

            # ---- pass 2
            pid = nc.vector.partition_id()
            for k, r0 in enumerate(R0S):
                adj_t = iop.tile([128, N], f32, tag="adj", name=f"adj{k}")
                nc.gpsimd.dma_start(adj_t[:], adj_in[r0 : r0 + 128, :])
                dvi = iop.tile([128, 1], f32, tag="dvi", name=f"dvi{k}")
                nc.gpsimd.dma_start(
                    dvi[:], dinvloc[0:1, r0 : r0 + 128].rearrange("o (p u) -> (o p) u", u=1)
                )
                A1 = a1p.tile([128, N], bf16, tag="A1", name=f"A1_{k}")
                nc.scalar.activation(A1[:], adj_t[:], Act.Relu, scale=dvi[:])
                Apad = apads[k % 2]
                nc.vector.tensor_tensor(
                    Apad[:, CB : CB + N], A1[:], colfac[:], Alu.mult
                )

                # stencil: t = A -0.25*(up+down+left+right) built on PE
                for q in range(4):
                    ps = psums[q % 2]
                    for cc in range(4):
                        c = 4 * q + cc
                        col = CB + 512 * c
                        out_ap = ps[0:126, 512 * cc : 512 * cc + 512]
                        nc.tensor.matmul(
                            out_ap, Mv[:], Apad[:, col : col + 512],
                            start=True, stop=False,
                        )
                        nc.tensor.matmul(
                            out_ap, NI[:], Apad[:, col - 1 : col + 511],
                            start=False, stop=False,
                        )
                        nc.tensor.matmul(
                            out_ap, NI[:], Apad[:, col + 1 : col + 513],
                            start=False, stop=True,
                        )
                    if q == 0:
                        nc.vector.memset(ps[0:126, 0:1], 0.0)
                    if q == 3:
                        nc.vector.memset(ps[0:126, 2047:2048], 0.0)
                    sabs = scrp.tile([126, 2048], bf16, tag="sabs", name=f"sabs{k}_{q}")
                    nc.scalar.activation(
                        sabs[:], ps[0:126, :], Act.Abs,
                        accum_out=acc[0:126, 4 * k + q : 4 * k + q + 1],
                    )

                # sum A^2 (row partials)
                sq = scrp.tile([128, N], bf16, tag="junk", name=f"sq{k}")
                nc.vector.scalar_tensor_tensor(
                    sq[:],
                    Apad[:, CB : CB + N],
                    1.0,
                    Apad[:, CB : CB + N],
                    Alu.bypass,
                    Alu.mult,
                    accum_out=acc[:, A2_COL + k : A2_COL + k + 1],
                )

                # band sum A*decay (row partials); dynamic slice by core id
                bscr = scrp.tile([128, BW], bf16, tag="junk", name=f"bscr{k}")
                nc.vector.scalar_tensor_tensor(
                    bscr[:],
                    Apad[:, bass.ds(pid * SH + (r0 + 63), BW)],
                    1.0,
                    decayb[:],
                    Alu.bypass,
                    Alu.mult,
                    accum_out=acc[:, BD_COL + k : BD_COL + k + 1],
                )

            acc2 = cp.tile([128, 64], f32)
            nc.vector.tensor_copy(acc2[:], acc[:])
            nc.gpsimd.dma_start(res_out[:], acc2[:])

    if legalize_waits is not None:
        legalize_waits(nc, verbose=True)
    nc.finalize()
    if legalize_waits is not None:
        from waitstrip import drop_broken_range_clear
        drop_broken_range_clear(nc, verbose=True)
    return nc


def _masks():
    """Row-ownership masks resolving overlap-tile double counting (per core)."""
    sm = np.zeros((NC, 128, 36), np.float64)
    rows = np.zeros((NC, 128, 9), np.float64)
    for c in range(NC):
        claimed_r = set()
        claimed_s = set()
        for k, r0 in enumerate(R0S):
            for p in range(128):
                L = r0 + p
                if 1 <= L <= 1024 and L not in claimed_r:
                    claimed_r.add(L)
                    rows[c, p, k] = 1.0
            for p in range(126):
                L = r0 + 1 + p           # stencil out row (local)
                g = SH * c - 1 + L       # global row
                if 1 <= L <= 1024 and 1 <= g <= N - 2 and L not in claimed_s:
                    claimed_s.add(L)
                    sm[c, p, 4 * k : 4 * k + 4] = 1.0
    return sm, rows


_SM_MASK, _ROW_MASK = _masks()


def _analytic_decay_sq():
    k = np.arange(1, N, dtype=np.float64)
    return N + 2.0 * np.sum((N - k) * np.exp(-2.0 * ALPHA * k))


_NC_CACHE = None


def kernel(adj):
    global _NC_CACHE
    adj = np.ascontiguousarray(np.asarray(adj), dtype=np.float32)
    assert adj.shape == (N, N)

    if _NC_CACHE is None:
        _NC_CACHE = _build_nc()
    nc = _NC_CACHE

    in_maps = []
    for c in range(NC):
        sl = np.zeros((LR, N), np.float32)
        lo = SH * c - 1
        src_lo = max(lo, 0)
        src_hi = min(lo + LR, N)
        sl[src_lo - lo : src_hi - lo, :] = adj[src_lo:src_hi]
        in_maps.append({"adj_sh": sl})

    res = run_bass_kernel_spmd(nc, in_maps, core_ids=list(range(NC)))
    s_sm = 0.0
    s_a2 = 0.0
    s_bd = 0.0
    for c in range(NC):
        o = res.results[c]["res"].astype(np.float64)
        s_sm += float((o[:, SM_COL : SM_COL + 36] * _SM_MASK[c]).sum())
        s_a2 += float((o[:, A2_COL : A2_COL + 9] * _ROW_MASK[c]).sum())
        s_bd += float((o[:, BD_COL : BD_COL + 9] * _ROW_MASK[c]).sum())

    d2 = _analytic_decay_sq()
    loss = (s_a2 - 2.0 * s_bd + d2) + LAM * s_sm + GAMMA * s_a2
    return np.float32(loss)


def run_profiled(adj):
    """Run once more with tracing enabled; returns BassKernelResults."""
    global _NC_CACHE
    adj = np.ascontiguousarray(np.asarray(adj), dtype=np.float32)
    if _NC_CACHE is None:
        _NC_CACHE = _build_nc()
    in_maps = []
    for c in range(NC):
        sl = np.zeros((LR, N), np.float32)
        lo = SH * c - 1
        src_lo = max(lo, 0)
        src_hi = min(lo + LR, N)
        sl[src_lo - lo : src_hi - lo, :] = adj[src_lo:src_hi]
        in_maps.append({"adj_sh": sl})
    return run_bass_kernel_spmd(
        _NC_CACHE, in_maps, core_ids=list(range(NC)), trace=True
    )
